# revision 1
# baseline (speedup 1.0000x reference)
"""Causal self-attention (GQA + RoPE) on 8 trn2 NeuronCores via Bass/Tile.

Sharding: tensor-parallel over heads for QKV-proj + attention (each core owns
2 q-heads and their shared kv-head); token-parallel for o_proj. The y
activations are exchanged between the two device kernels on the host (a pure
gather/reslice - all FLOPs run on device).

Kernel A is a fused per-batch pipeline: for each batch, project Q/K/V for its
4 token chunks (Q/K/V stay in SBUF, no DRAM round trip), apply RoPE, then run
causal attention for the core's 2 heads. Softmax normalization is deferred
(sum + unnormalized y are banked to SBUF; the reciprocal/scale runs two
chunks later) so the PE never stalls on the softmax tail.

Numerics: float32r (fp32 stored, ~21-bit matmul datapath) everywhere on the
PE; fp32 PSUM accumulation; softmax without max-subtraction (|scores/sqrt(d)|
<= ~7 for this input distribution, exp is safe in fp32).

Shapes hardcoded for B=4, T=2048, D=2048, 16 heads x 128, 4 kv heads x 128.
"""
import numpy as np

import concourse.bacc as bacc
import concourse.mybir as mybir
from concourse.tile import TileContext
from concourse.bass_utils import run_bass_kernel_spmd

N_CORES = 8
B, T, D = 4, 2048, 2048
N_HEAD, N_KV, HD = 16, 4, 128
NTOK = B * T                      # 8192
CHUNK = 512
QC_PER_B = T // CHUNK             # 4
TOK_PER_CORE = NTOK // N_CORES    # 1024
SCALE = float(1.0 / np.sqrt(128.0))
ROPE_THETA = 10000.0

F32 = mybir.dt.float32
F32R = mybir.dt.float32r


def _round_f32r(a):
    """Round fp32 ndarray to the fp32r grid (1+8+11 bits, RNE)."""
    u = np.ascontiguousarray(a, dtype=np.float32).view(np.uint32)
    add = np.uint32(0x7FF) + ((u >> np.uint32(12)) & np.uint32(1))
    u = (u + add) & np.uint32(0xFFFFF000)
    return u.view(np.float32)


def build_kernel_a():
    nc = bacc.Bacc("TRN2", target_bir_lowering=False, debug=False,
                   num_devices=N_CORES, name="attn_a")
    xT = nc.dram_tensor("xT", [D, NTOK], F32R, kind="ExternalInput")
    wq = nc.dram_tensor("wq", [128, 16, 256], F32R, kind="ExternalInput")
    wk = nc.dram_tensor("wk", [128, 16, 128], F32R, kind="ExternalInput")
    wv = nc.dram_tensor("wv", [128, 16, 128], F32R, kind="ExternalInput")
    cosT = nc.dram_tensor("cosT", [128, T], F32, kind="ExternalInput")
    sinM = nc.dram_tensor("sinM", [128, T], F32, kind="ExternalInput")
    maskW = nc.dram_tensor("maskW", [128, 896], F32R, kind="ExternalInput")
    ident_in = nc.dram_tensor("ident_in", [128, 128], F32R, kind="ExternalInput")
    ones_in = nc.dram_tensor("ones_in", [128, 1], F32R, kind="ExternalInput")
    onesr_in = nc.dram_tensor("onesr_in", [1, 128], F32R, kind="ExternalInput")
    y = nc.dram_tensor("y", [256, NTOK], F32R, kind="ExternalOutput")

    xT_r = xT.rearrange("(kt p) t -> p kt t", p=128)   # [128, 16, NTOK]
    wq_r, wk_r, wv_r = wq, wk, wv

    with TileContext(nc) as tc:
        with tc.tile_pool(name="wpool", bufs=1) as wpool, \
             tc.tile_pool(name="xpool", bufs=6) as xpool, \
             tc.tile_pool(name="tpool", bufs=2) as tpool, \
             tc.tile_pool(name="qkv", bufs=2) as qkv, \
             tc.tile_pool(name="ep", bufs=3) as ep, \
             tc.tile_pool(name="yu", bufs=4) as yu, \
             tc.tile_pool(name="su", bufs=4) as su, \
             tc.tile_pool(name="yp", bufs=2) as yp, \
             tc.tile_pool(name="psum", bufs=1, space="PSUM") as pp:
            # DMA issue order = HWDGE service order: the first proj (K of
            # batch 0) needs wk + the first x chunk, so those go first.
            wk_sb = wpool.tile([128, 16, 128], F32R)
            nc.sync.dma_start(out=wk_sb[:], in_=wk_r[:])
            xq0 = []
            for qtr in range(4):
                t = xpool.tile([128, 4, CHUNK], F32R, name="xq")
                nc.sync.dma_start(out=t[:], in_=xT_r[:, 4 * qtr:4 * qtr + 4, 0:CHUNK])
                xq0.append(t)
            cos_sb = wpool.tile([128, T], F32)
            nc.sync.dma_start(out=cos_sb[:], in_=cosT[:])
            sin_sb = wpool.tile([128, T], F32)
            nc.sync.dma_start(out=sin_sb[:], in_=sinM[:])
            wq_sb = wpool.tile([128, 16, 256], F32R)
            nc.sync.dma_start(out=wq_sb[:], in_=wq_r[:])
            wv_sb = wpool.tile([128, 16, 128], F32R)
            nc.sync.dma_start(out=wv_sb[:], in_=wv_r[:])
            id_sb = wpool.tile([128, 128], F32R)
            nc.sync.dma_start(out=id_sb[:], in_=ident_in[:])
            mask_sb = wpool.tile([128, 896], F32R)
            nc.sync.dma_start(out=mask_sb[:], in_=maskW[:])
            ones_sb = wpool.tile([128, 1], F32R)
            nc.sync.dma_start(out=ones_sb[:], in_=ones_in[:])
            onesr_sb = wpool.tile([1, 128], F32R)
            nc.sync.dma_start(out=onesr_sb[:], in_=onesr_in[:])

            pending = []

            def flush_one():
                # rrow already holds 1/sum (via ACT ln+exp); broadcast it over
                # partitions with a K=1 matmul, then scale the banked y.
                y_u, rrow, h, b, qc = pending.pop(0)
                col0 = b * T + qc * CHUNK
                b_ps = pp.tile([128, CHUNK], F32, name="b_ps", bufs=1)
                nc.tensor.matmul(b_ps[:], onesr_sb[:], rrow[:], start=True, stop=True)
                y_sb = yp.tile([128, CHUNK], F32R, name="y_sb")
                nc.vector.tensor_mul(y_sb[:], y_u[:], b_ps[:])
                nc.sync.dma_start(out=y[h * 128:(h + 1) * 128, col0:col0 + CHUNK],
                                  in_=y_sb[:])

            for b in range(B):
                # ---- projections + rope for batch b ----
                qb = [qkv.tile([128, T], F32R, name=f"qb{h}") for h in range(2)]
                kb = qkv.tile([128, T], F32R, name="kb")
                vtb = qkv.tile([128, 16, 128], F32R, name="vtb")
                for cc in range(QC_PER_B):
                    c0 = b * T + cc * CHUNK
                    tcol = cc * CHUNK
                    if b == 0 and cc == 0:
                        xq = xq0
                    else:
                        xq = []
                        for qtr in range(4):
                            t = xpool.tile([128, 4, CHUNK], F32R, name="xq")
                            nc.sync.dma_start(
                                out=t[:], in_=xT_r[:, 4 * qtr:4 * qtr + 4, c0:c0 + CHUNK])
                            xq.append(t)

                    def proj(w_sb, off):
                        ps = pp.tile([128, CHUNK], F32, name="ps", bufs=2)
                        for kt in range(16):
                            nc.tensor.matmul(ps[:], w_sb[:, kt, off:off + 128],
                                             xq[kt // 4][:, kt % 4, :],
                                             start=(kt == 0), stop=(kt == 15))
                        return ps

                    def rope(ps, dst):
                        t1 = tpool.tile([128, CHUNK], F32, name="t1")
                        t2 = tpool.tile([128, CHUNK], F32, name="t2")
                        nc.vector.tensor_mul(t1[:], ps[:], cos_sb[:, tcol:tcol + CHUNK])
                        nc.vector.tensor_mul(t2[0:64, :], ps[64:128, :],
                                             sin_sb[0:64, tcol:tcol + CHUNK])
                        nc.vector.tensor_mul(t2[64:128, :], ps[0:64, :],
                                             sin_sb[64:128, tcol:tcol + CHUNK])
                        nc.vector.tensor_add(dst, t1[:], t2[:])

                    rope(proj(wk_sb, 0), kb[:, tcol:tcol + CHUNK])
                    rope(proj(wq_sb, 0), qb[0][:, tcol:tcol + CHUNK])
                    rope(proj(wq_sb, 128), qb[1][:, tcol:tcol + CHUNK])

                    ps_v = proj(wv_sb, 0)
                    vtmp = tpool.tile([128, CHUNK], F32R, name="vtmp")
                    nc.scalar.copy(vtmp[:], ps_v[:])
                    for j in range(4):
                        pt = pp.tile([128, 128], F32R, name="s_ps", bufs=2)
                        nc.tensor.transpose(pt[:], vtmp[:, j * 128:(j + 1) * 128], id_sb[:])
                        nc.scalar.copy(vtb[:, 4 * cc + j, :], pt[:])

                # ---- attention for batch b ----
                for h in range(2):
                    for qc in range(QC_PER_B):
                        while len(pending) > 2:
                            flush_one()
                        nkt = 4 * qc + 4
                        y_ps = pp.tile([128, CHUNK], F32, name="y_ps", bufs=2)
                        sum_ps = pp.tile([1, CHUNK], F32, name="sum_ps", bufs=1)
                        for kt in range(nkt):
                            s_ps = pp.tile([128, CHUNK], F32, name="s_ps", bufs=2)
                            nc.tensor.matmul(s_ps[:], kb[:, kt * 128:(kt + 1) * 128],
                                             qb[h][:, qc * CHUNK:(qc + 1) * CHUNK],
                                             start=True, stop=True)
                            e_sb = ep.tile([128, CHUNK], F32R, name="e_sb")
                            nc.scalar.activation(e_sb[:], s_ps[:],
                                                 mybir.ActivationFunctionType.Exp,
                                                 bias=0.0, scale=SCALE)
                            delta = kt * 128 - qc * CHUNK
                            if delta >= 0:
                                off = 384 - delta
                                nc.vector.tensor_mul(e_sb[:], e_sb[:],
                                                     mask_sb[:, off:off + CHUNK])
                            nc.tensor.matmul(sum_ps[:], ones_sb[:], e_sb[:],
                                             start=(kt == 0), stop=(kt == nkt - 1))
                            nc.tensor.matmul(y_ps[:], vtb[:, kt, :], e_sb[:],
                                             start=(kt == 0), stop=(kt == nkt - 1))
                        y_u = yu.tile([128, CHUNK], F32, name="y_u")
                        nc.scalar.copy(y_u[:], y_ps[:])
                        # 1/sum: approx-NR reciprocal on DVE (~18 bits, 5x
                        # faster than the exact ucode reciprocal), then a
                        # dtype-cast copy on ACT so the broadcast matmul can
                        # consume it as f32r.
                        lrow = su.tile([1, CHUNK], F32, name="lrow", bufs=2)
                        nc.vector.reciprocal_approx_fast(out=lrow[:], in_=sum_ps[:])
                        rrow = su.tile([1, CHUNK], F32R, name="rrow")
                        nc.scalar.copy(rrow[:], lrow[:])
                        pending.append((y_u, rrow, h, b, qc))
            while pending:
                flush_one()
    nc.compile()
    return nc


def build_kernel_b():
    nc = bacc.Bacc("TRN2", target_bir_lowering=False, debug=False,
                   num_devices=N_CORES, name="attn_b")
    ya = nc.dram_tensor("ya", [128, 16, TOK_PER_CORE], F32R, kind="ExternalInput")
    wo = nc.dram_tensor("wo", [128, 16, D], F32R, kind="ExternalInput")
    outp = nc.dram_tensor("outp", [TOK_PER_CORE, D], F32, kind="ExternalOutput")
    ya_r = ya
    wo_r = wo
    NTT = TOK_PER_CORE // 128          # 8
    with TileContext(nc) as tc:
        with tc.tile_pool(name="yap", bufs=1) as yap, \
             tc.tile_pool(name="wop", bufs=2) as wop, \
             tc.tile_pool(name="obp", bufs=3) as obp, \
             tc.tile_pool(name="pb", bufs=4, space="PSUM") as pb:
            # first oc's weights go first so the PE can start after ~3 MB of DMA
            wlo0 = wop.tile([128, 8, 512], F32R, name="wlo")
            nc.sync.dma_start(out=wlo0[:], in_=wo_r[:, 0:8, 0:512])
            ya_t = []
            for tt in range(NTT):
                t = yap.tile([128, 16, 128], F32R, name=f"yat{tt}")
                nc.sync.dma_start(out=t[:], in_=ya_r[:, :, tt * 128:(tt + 1) * 128])
                ya_t.append(t)
                if tt == 0:
                    whi0 = wop.tile([128, 8, 512], F32R, name="whi")
                    nc.sync.dma_start(out=whi0[:], in_=wo_r[:, 8:16, 0:512])
            for oc in range(4):
                if oc == 0:
                    wlo, whi = wlo0, whi0
                else:
                    wlo = wop.tile([128, 8, 512], F32R, name="wlo")
                    nc.sync.dma_start(out=wlo[:], in_=wo_r[:, 0:8, oc * 512:(oc + 1) * 512])
                    whi = wop.tile([128, 8, 512], F32R, name="whi")
                    nc.sync.dma_start(out=whi[:], in_=wo_r[:, 8:16, oc * 512:(oc + 1) * 512])
                for tt in range(NTT):
                    ps = pb.tile([128, 512], F32, name="ps")
                    for kt in range(16):
                        w = wlo if kt < 8 else whi
                        nc.tensor.matmul(ps[:], ya_t[tt][:, kt, :], w[:, kt % 8, :],
                                         start=(kt == 0), stop=(kt == 15))
                    ob = obp.tile([128, 512], F32, name="ob")
                    nc.scalar.copy(ob[:], ps[:])
                    nc.sync.dma_start(
                        out=outp[tt * 128:(tt + 1) * 128, oc * 512:(oc + 1) * 512],
                        in_=ob[:])
    nc.compile()
    return nc


_cache = {}


def _get_kernels():
    if "a" not in _cache:
        _cache["a"] = build_kernel_a()
        _cache["b"] = build_kernel_b()
    return _cache["a"], _cache["b"]


def _to_pkto(w):
    # (D, O) -> (128, D//128, O): partition-major layout matching SBUF tiles
    Dd, O = w.shape
    return np.ascontiguousarray(w.reshape(Dd // 128, 128, O).transpose(1, 0, 2))


def _prep_inputs(x, position_ids, Wq, Wk, Wv, Wo):
    x = np.ascontiguousarray(np.asarray(x), dtype=np.float32)
    pos = np.asarray(position_ids).astype(np.float32)
    Wq = np.asarray(Wq, dtype=np.float32)
    Wk = np.asarray(Wk, dtype=np.float32)
    Wv = np.asarray(Wv, dtype=np.float32)
    Wo = np.asarray(Wo, dtype=np.float32)

    xT = _round_f32r(x.reshape(NTOK, D).T)

    inv = (1.0 / (ROPE_THETA ** (np.arange(0, HD, 2, dtype=np.float32) / HD))).astype(np.float32)
    freqs = np.outer(pos, inv).astype(np.float32)          # (T, 64)
    emb = np.concatenate([freqs, freqs], axis=1)           # (T, 128)
    cosT = np.ascontiguousarray(np.cos(emb).T)             # (128, T)
    sinT = np.sin(emb).T
    sign = np.where(np.arange(128) < 64, -1.0, 1.0).astype(np.float32)
    sinM = np.ascontiguousarray(sinT * sign[:, None])

    # wide causal mask: maskW[p, u] = 1 iff p <= u - 384
    p_idx = np.arange(128)[:, None]
    u_idx = np.arange(896)[None, :]
    maskW = (p_idx <= u_idx - 384).astype(np.float32)

    ident = np.eye(128, dtype=np.float32)
    ones_c = np.ones((128, 1), np.float32)
    ones_r = np.ones((1, 128), np.float32)

    wo_r = _to_pkto(_round_f32r(Wo))

    in_maps_a = []
    for c in range(N_CORES):
        g = c // 2
        in_maps_a.append({
            "xT": xT,
            "wq": _to_pkto(_round_f32r(Wq[:, 256 * c:256 * c + 256])),
            "wk": _to_pkto(_round_f32r(Wk[:, 128 * g:128 * g + 128])),
            "wv": _to_pkto(_round_f32r(Wv[:, 128 * g:128 * g + 128])),
            "cosT": cosT,
            "sinM": sinM,
            "maskW": maskW,
            "ident_in": ident,
            "ones_in": ones_c,
            "onesr_in": ones_r,
        })
    return in_maps_a, wo_r


def kernel(x, position_ids, Wq, Wk, Wv, Wo, _trace=False, _trace_kwargs=None):
    nca, ncb = _get_kernels()
    in_maps_a, wo_r = _prep_inputs(x, position_ids, Wq, Wk, Wv, Wo)

    kw = dict(trace=True, **(_trace_kwargs or {})) if _trace else {}
    res_a = run_bass_kernel_spmd(nca, in_maps_a, list(range(N_CORES)), **kw)
    y_allT = np.concatenate([res_a.results[c]["y"] for c in range(N_CORES)], axis=0)

    ya_p = y_allT.reshape(16, 128, NTOK).transpose(1, 0, 2)   # (128, 16, NTOK)
    in_maps_b = [{
        "ya": np.ascontiguousarray(ya_p[:, :, TOK_PER_CORE * c:TOK_PER_CORE * (c + 1)]),
        "wo": wo_r,
    } for c in range(N_CORES)]
    res_b = run_bass_kernel_spmd(ncb, in_maps_b, list(range(N_CORES)), **kw)
    out = np.concatenate([res_b.results[c]["outp"] for c in range(N_CORES)], axis=0)
    out = out.reshape(B, T, D).astype(np.float32)
    if _trace:
        return out, res_a, res_b
    return out



# revision 2
# speedup vs baseline: 1.2620x; 1.2620x over previous
"""Causal self-attention (GQA + RoPE) on 8 trn2 NeuronCores via Bass/Tile.

Sharding: core c = (kv-group g=c//2, batch-pair bp=c%2). Each core projects
Q (4 heads = one GQA group) / K / V for its 2 batches only -- no duplicated
K/V work across cores -- then runs causal attention for those 4 heads; o_proj
runs token-parallel in a second kernel. The y activations are exchanged
between the two device kernels on the host (a pure gather/reslice).

Kernel A is a fused per-batch pipeline: project Q/K/V for the batch's 4
token chunks (Q/K/V stay in SBUF as bf16), apply RoPE, then run causal
attention. The softmax sum is NOT computed with per-key-block ones-matmuls
on the PE (that costs a full 512-row pass each); instead the exp tiles are
accumulated on the DVE in bf16 (4x perf mode) and a single ones-matmul per
q-chunk does the final partition reduce. Softmax normalization is deferred
(sum + unnormalized y are banked to SBUF; the reciprocal/scale runs two
chunks later) so the PE never stalls on the softmax tail.

Numerics: x & projection weights in float32r (fp32 stored, ~21-bit matmul
datapath); q/k/v/exp/y activations in bf16 (same PE rate as f32r, 2x DMA and
4x DVE); fp32 PSUM accumulation everywhere; softmax without max-subtraction
(|scores| small for this input distribution).

Shapes hardcoded for B=4, T=2048, D=2048, 16 heads x 128, 4 kv heads x 128.
"""
import numpy as np
import ml_dtypes

import concourse.bacc as bacc
import concourse.mybir as mybir
from concourse.tile import TileContext
from concourse.bass_utils import run_bass_kernel_spmd

N_CORES = 8
B, T, D = 4, 2048, 2048
N_HEAD, N_KV, HD = 16, 4, 128
NTOK = B * T                      # 8192
CHUNK = 512
QC_PER_B = T // CHUNK             # 4
TOK_PER_CORE = NTOK // N_CORES    # 1024 (kernel B)
TOKA = 2 * T                      # 4096 tokens per core in kernel A
SCALE = float(1.0 / np.sqrt(128.0))
ROPE_THETA = 10000.0

F32 = mybir.dt.float32
F32R = mybir.dt.float32r
BF16 = mybir.dt.bfloat16
NP_BF16 = ml_dtypes.bfloat16


def _round_f32r(a):
    """Round fp32 ndarray to the fp32r grid (1+8+11 bits, RNE)."""
    u = np.ascontiguousarray(a, dtype=np.float32).view(np.uint32)
    add = np.uint32(0x7FF) + ((u >> np.uint32(12)) & np.uint32(1))
    u = (u + add) & np.uint32(0xFFFFF000)
    return u.view(np.float32)


def build_kernel_a():
    nc = bacc.Bacc("TRN2", target_bir_lowering=False, debug=False,
                   num_devices=N_CORES, name="attn_a")
    xT = nc.dram_tensor("xT", [D, TOKA], F32R, kind="ExternalInput")
    wq = nc.dram_tensor("wq", [128, 16, 512], F32R, kind="ExternalInput")
    wk = nc.dram_tensor("wk", [128, 16, 128], F32R, kind="ExternalInput")
    wv = nc.dram_tensor("wv", [128, 16, 128], F32R, kind="ExternalInput")
    cosT = nc.dram_tensor("cosT", [128, T], F32, kind="ExternalInput")
    sinM = nc.dram_tensor("sinM", [128, T], F32, kind="ExternalInput")
    maskW = nc.dram_tensor("maskW", [128, 896], BF16, kind="ExternalInput")
    ident_in = nc.dram_tensor("ident_in", [128, 128], BF16, kind="ExternalInput")
    ones_in = nc.dram_tensor("ones_in", [128, 1], BF16, kind="ExternalInput")
    onesr_in = nc.dram_tensor("onesr_in", [1, 128], BF16, kind="ExternalInput")
    y = nc.dram_tensor("y", [512, TOKA], BF16, kind="ExternalOutput")

    xT_r = xT.rearrange("(kt p) t -> p kt t", p=128)   # [128, 16, TOKA]

    with TileContext(nc) as tc:
        with tc.tile_pool(name="wpool", bufs=1) as wpool, \
             tc.tile_pool(name="xpool", bufs=6) as xpool, \
             tc.tile_pool(name="tpool", bufs=2) as tpool, \
             tc.tile_pool(name="qkv", bufs=2) as qkv, \
             tc.tile_pool(name="ep", bufs=4) as ep, \
             tc.tile_pool(name="sp", bufs=2) as sp, \
             tc.tile_pool(name="yu", bufs=4) as yu, \
             tc.tile_pool(name="su", bufs=4) as su, \
             tc.tile_pool(name="yp", bufs=2) as yp, \
             tc.tile_pool(name="psum", bufs=1, space="PSUM") as pp:
            # DMA issue order = HWDGE service order: the first proj (K of
            # batch 0) needs wk + the first x chunk, so those go first.
            wk_sb = wpool.tile([128, 16, 128], F32R)
            nc.sync.dma_start(out=wk_sb[:], in_=wk[:])
            xq0 = []
            for qtr in range(4):
                t = xpool.tile([128, 4, CHUNK], F32R, name="xq")
                nc.sync.dma_start(out=t[:], in_=xT_r[:, 4 * qtr:4 * qtr + 4, 0:CHUNK])
                xq0.append(t)
            cos_sb = wpool.tile([128, T], F32)
            nc.sync.dma_start(out=cos_sb[:], in_=cosT[:])
            sin_sb = wpool.tile([128, T], F32)
            nc.sync.dma_start(out=sin_sb[:], in_=sinM[:])
            wq_sb = wpool.tile([128, 16, 512], F32R)
            nc.sync.dma_start(out=wq_sb[:], in_=wq[:])
            wv_sb = wpool.tile([128, 16, 128], F32R)
            nc.sync.dma_start(out=wv_sb[:], in_=wv[:])
            id_sb = wpool.tile([128, 128], BF16)
            nc.sync.dma_start(out=id_sb[:], in_=ident_in[:])
            mask_sb = wpool.tile([128, 896], BF16)
            nc.sync.dma_start(out=mask_sb[:], in_=maskW[:])
            ones_sb = wpool.tile([128, 1], BF16)
            nc.sync.dma_start(out=ones_sb[:], in_=ones_in[:])
            onesr_sb = wpool.tile([1, 128], BF16)
            nc.sync.dma_start(out=onesr_sb[:], in_=onesr_in[:])

            pending = []

            def flush_one():
                # rrow already holds 1/sum; broadcast it over partitions with
                # a K=1 matmul, then scale the banked y.
                y_u, rrow, h, b, qc = pending.pop(0)
                col0 = b * T + qc * CHUNK
                b_ps = pp.tile([128, CHUNK], F32, name="b_ps", bufs=1)
                nc.tensor.matmul(b_ps[:], onesr_sb[:], rrow[:], start=True, stop=True)
                y_sb = yp.tile([128, CHUNK], BF16, name="y_sb")
                nc.vector.tensor_mul(y_sb[:], y_u[:], b_ps[:])
                nc.sync.dma_start(out=y[h * 128:(h + 1) * 128, col0:col0 + CHUNK],
                                  in_=y_sb[:])

            for b in range(2):
                # ---- projections + rope for batch b ----
                qb = [qkv.tile([128, T], BF16, name=f"qb{h}") for h in range(4)]
                kb = qkv.tile([128, T], BF16, name="kb")
                vtb = qkv.tile([128, 16, 128], BF16, name="vtb")
                for cc in range(QC_PER_B):
                    c0 = b * T + cc * CHUNK
                    tcol = cc * CHUNK
                    if b == 0 and cc == 0:
                        xq = xq0
                    else:
                        xq = []
                        for qtr in range(4):
                            t = xpool.tile([128, 4, CHUNK], F32R, name="xq")
                            nc.sync.dma_start(
                                out=t[:], in_=xT_r[:, 4 * qtr:4 * qtr + 4, c0:c0 + CHUNK])
                            xq.append(t)

                    def proj(w_sb, off):
                        ps = pp.tile([128, CHUNK], F32, name="ps", bufs=2)
                        for kt in range(16):
                            nc.tensor.matmul(ps[:], w_sb[:, kt, off:off + 128],
                                             xq[kt // 4][:, kt % 4, :],
                                             start=(kt == 0), stop=(kt == 15))
                        return ps

                    def rope(ps, dst):
                        t1 = tpool.tile([128, CHUNK], F32, name="t1")
                        t2 = tpool.tile([128, CHUNK], F32, name="t2")
                        nc.vector.tensor_mul(t1[:], ps[:], cos_sb[:, tcol:tcol + CHUNK])
                        nc.vector.tensor_mul(t2[0:64, :], ps[64:128, :],
                                             sin_sb[0:64, tcol:tcol + CHUNK])
                        nc.vector.tensor_mul(t2[64:128, :], ps[0:64, :],
                                             sin_sb[64:128, tcol:tcol + CHUNK])
                        nc.vector.tensor_add(dst, t1[:], t2[:])

                    rope(proj(wk_sb, 0), kb[:, tcol:tcol + CHUNK])
                    for h in range(4):
                        rope(proj(wq_sb, 128 * h), qb[h][:, tcol:tcol + CHUNK])

                    ps_v = proj(wv_sb, 0)
                    vtmp = tpool.tile([128, CHUNK], BF16, name="vtmp")
                    nc.scalar.copy(vtmp[:], ps_v[:])
                    for j in range(4):
                        pt = pp.tile([128, 128], BF16, name="s_ps", bufs=2)
                        nc.tensor.transpose(pt[:], vtmp[:, j * 128:(j + 1) * 128], id_sb[:])
                        nc.scalar.copy(vtb[:, 4 * cc + j, :], pt[:])

                # ---- attention for batch b ----
                for h in range(4):
                    for qc in range(QC_PER_B):
                        while len(pending) > 2:
                            flush_one()
                        nkt = 4 * qc + 4
                        y_ps = pp.tile([128, CHUNK], F32, name="y_ps", bufs=2)
                        esum = sp.tile([128, CHUNK], BF16, name="esum")
                        e_prev = None
                        for kt in range(nkt):
                            s_ps = pp.tile([128, CHUNK], F32, name="s_ps", bufs=2)
                            nc.tensor.matmul(s_ps[:], kb[:, kt * 128:(kt + 1) * 128],
                                             qb[h][:, qc * CHUNK:(qc + 1) * CHUNK],
                                             start=True, stop=True)
                            e_sb = ep.tile([128, CHUNK], BF16, name="e_sb")
                            nc.scalar.activation(e_sb[:], s_ps[:],
                                                 mybir.ActivationFunctionType.Exp,
                                                 bias=0.0, scale=SCALE)
                            delta = kt * 128 - qc * CHUNK
                            if delta >= 0:
                                off = 384 - delta
                                nc.vector.tensor_mul(e_sb[:], e_sb[:],
                                                     mask_sb[:, off:off + CHUNK])
                            # bf16 DVE accumulation of the softmax sum (the
                            # partition reduce happens once per q-chunk below)
                            if kt == 0:
                                e_prev = e_sb
                            elif kt == 1:
                                nc.vector.tensor_add(esum[:], e_prev[:], e_sb[:])
                            else:
                                nc.vector.tensor_add(esum[:], esum[:], e_sb[:])
                            nc.tensor.matmul(y_ps[:], vtb[:, kt, :], e_sb[:],
                                             start=(kt == 0), stop=(kt == nkt - 1))
                        sum_ps = pp.tile([1, CHUNK], F32, name="sum_ps", bufs=1)
                        nc.tensor.matmul(sum_ps[:], ones_sb[:], esum[:],
                                         start=True, stop=True)
                        y_u = yu.tile([128, CHUNK], BF16, name="y_u")
                        nc.scalar.copy(y_u[:], y_ps[:])
                        # 1/sum: approx-NR reciprocal on DVE (~18 bits, 5x
                        # faster than the exact ucode reciprocal), then a
                        # dtype-cast copy on ACT so the broadcast matmul can
                        # consume it as bf16.
                        lrow = su.tile([1, CHUNK], F32, name="lrow", bufs=2)
                        nc.vector.reciprocal_approx_fast(out=lrow[:], in_=sum_ps[:])
                        rrow = su.tile([1, CHUNK], BF16, name="rrow")
                        nc.scalar.copy(rrow[:], lrow[:])
                        pending.append((y_u, rrow, h, b, qc))
            while pending:
                flush_one()
    nc.compile()
    return nc


def build_kernel_b():
    nc = bacc.Bacc("TRN2", target_bir_lowering=False, debug=False,
                   num_devices=N_CORES, name="attn_b")
    ya = nc.dram_tensor("ya", [128, 16, TOK_PER_CORE], BF16, kind="ExternalInput")
    wo = nc.dram_tensor("wo", [128, 16, D], BF16, kind="ExternalInput")
    outp = nc.dram_tensor("outp", [TOK_PER_CORE, D], F32, kind="ExternalOutput")
    ya_r = ya
    wo_r = wo
    NTT = TOK_PER_CORE // 128          # 8
    with TileContext(nc) as tc:
        with tc.tile_pool(name="yap", bufs=1) as yap, \
             tc.tile_pool(name="wop", bufs=2) as wop, \
             tc.tile_pool(name="obp", bufs=3) as obp, \
             tc.tile_pool(name="pb", bufs=4, space="PSUM") as pb:
            # first oc's weights go first so the PE can start after ~1.5 MB of DMA
            wlo0 = wop.tile([128, 8, 512], BF16, name="wlo")
            nc.sync.dma_start(out=wlo0[:], in_=wo_r[:, 0:8, 0:512])
            ya_t = []
            for tt in range(NTT):
                t = yap.tile([128, 16, 128], BF16, name=f"yat{tt}")
                nc.sync.dma_start(out=t[:], in_=ya_r[:, :, tt * 128:(tt + 1) * 128])
                ya_t.append(t)
                if tt == 0:
                    whi0 = wop.tile([128, 8, 512], BF16, name="whi")
                    nc.sync.dma_start(out=whi0[:], in_=wo_r[:, 8:16, 0:512])
            for oc in range(4):
                if oc == 0:
                    wlo, whi = wlo0, whi0
                else:
                    wlo = wop.tile([128, 8, 512], BF16, name="wlo")
                    nc.sync.dma_start(out=wlo[:], in_=wo_r[:, 0:8, oc * 512:(oc + 1) * 512])
                    whi = wop.tile([128, 8, 512], BF16, name="whi")
                    nc.sync.dma_start(out=whi[:], in_=wo_r[:, 8:16, oc * 512:(oc + 1) * 512])
                for tt in range(NTT):
                    ps = pb.tile([128, 512], F32, name="ps")
                    for kt in range(16):
                        w = wlo if kt < 8 else whi
                        nc.tensor.matmul(ps[:], ya_t[tt][:, kt, :], w[:, kt % 8, :],
                                         start=(kt == 0), stop=(kt == 15))
                    ob = obp.tile([128, 512], F32, name="ob")
                    nc.scalar.copy(ob[:], ps[:])
                    nc.sync.dma_start(
                        out=outp[tt * 128:(tt + 1) * 128, oc * 512:(oc + 1) * 512],
                        in_=ob[:])
    nc.compile()
    return nc


_cache = {}


def _get_kernels():
    if "a" not in _cache:
        _cache["a"] = build_kernel_a()
        _cache["b"] = build_kernel_b()
    return _cache["a"], _cache["b"]


def _to_pkto(w):
    # (D, O) -> (128, D//128, O): partition-major layout matching SBUF tiles
    Dd, O = w.shape
    return np.ascontiguousarray(w.reshape(Dd // 128, 128, O).transpose(1, 0, 2))


def _prep_inputs(x, position_ids, Wq, Wk, Wv, Wo):
    x = np.ascontiguousarray(np.asarray(x), dtype=np.float32)
    pos = np.asarray(position_ids).astype(np.float32)
    Wq = np.asarray(Wq, dtype=np.float32)
    Wk = np.asarray(Wk, dtype=np.float32)
    Wv = np.asarray(Wv, dtype=np.float32)
    Wo = np.asarray(Wo, dtype=np.float32)

    # per-batch-pair x slices, transposed: [D, 4096]
    xT_bp = [
        _round_f32r(x[2 * bp:2 * bp + 2].reshape(TOKA, D).T) for bp in range(2)
    ]

    inv = (1.0 / (ROPE_THETA ** (np.arange(0, HD, 2, dtype=np.float32) / HD))).astype(np.float32)
    freqs = np.outer(pos, inv).astype(np.float32)          # (T, 64)
    emb = np.concatenate([freqs, freqs], axis=1)           # (T, 128)
    cosT = np.ascontiguousarray(np.cos(emb).T)             # (128, T)
    sinT = np.sin(emb).T
    sign = np.where(np.arange(128) < 64, -1.0, 1.0).astype(np.float32)
    sinM = np.ascontiguousarray(sinT * sign[:, None])

    # wide causal mask: maskW[p, u] = 1 iff p <= u - 384
    p_idx = np.arange(128)[:, None]
    u_idx = np.arange(896)[None, :]
    maskW = (p_idx <= u_idx - 384).astype(NP_BF16)

    ident = np.eye(128, dtype=NP_BF16)
    ones_c = np.ones((128, 1), NP_BF16)
    ones_r = np.ones((1, 128), NP_BF16)

    wo_r = _to_pkto(Wo).astype(NP_BF16)

    in_maps_a = []
    for c in range(N_CORES):
        g, bp = c // 2, c % 2
        in_maps_a.append({
            "xT": xT_bp[bp],
            "wq": _to_pkto(_round_f32r(Wq[:, 512 * g:512 * g + 512])),
            "wk": _to_pkto(_round_f32r(Wk[:, 128 * g:128 * g + 128])),
            "wv": _to_pkto(_round_f32r(Wv[:, 128 * g:128 * g + 128])),
            "cosT": cosT,
            "sinM": sinM,
            "maskW": maskW,
            "ident_in": ident,
            "ones_in": ones_c,
            "onesr_in": ones_r,
        })
    return in_maps_a, wo_r


def kernel(x, position_ids, Wq, Wk, Wv, Wo, _trace=False, _trace_kwargs=None):
    nca, ncb = _get_kernels()
    in_maps_a, wo_r = _prep_inputs(x, position_ids, Wq, Wk, Wv, Wo)

    kw = dict(trace=True, **(_trace_kwargs or {})) if _trace else {}
    res_a = run_bass_kernel_spmd(nca, in_maps_a, list(range(N_CORES)), **kw)
    # core c=(g,bp) emitted y rows for heads 4g..4g+3, tokens of batches
    # {2bp, 2bp+1}: stack per batch-pair into the full [2048, 4096] yT
    yT_half = [
        np.concatenate([res_a.results[2 * g + bp]["y"] for g in range(4)], axis=0)
        for bp in range(2)
    ]

    in_maps_b = []
    for d in range(N_CORES):
        bp, off = d // 4, (d % 4) * TOK_PER_CORE
        ya_d = yT_half[bp][:, off:off + TOK_PER_CORE]
        ya_p = ya_d.reshape(16, 128, TOK_PER_CORE).transpose(1, 0, 2)
        in_maps_b.append({"ya": np.ascontiguousarray(ya_p), "wo": wo_r})
    res_b = run_bass_kernel_spmd(ncb, in_maps_b, list(range(N_CORES)), **kw)
    out = np.concatenate([res_b.results[c]["outp"] for c in range(N_CORES)], axis=0)
    out = out.reshape(B, T, D).astype(np.float32)
    if _trace:
        return out, res_a, res_b
    return out
